# revision 7
# baseline (speedup 1.0000x reference)
"""Differentiable-FK forward kernel for Trainium2 (8 NeuronCores, data-parallel).

Problem: batch B=131072 of kinematic chains (63 bodies: world, free root,
61 hinges), 16 sites gathered from bodies. Output [B, 16, 3] site positions.

Strategy: pure data-parallel across 8 cores (16384 rows each). Per core the
batch is laid out as [128 partitions x 128 free]. The hinge chain is evaluated
sequentially (61 quaternion composes) on the Vector engine with the Scalar
engine supplying bulk sin/cos. Positions use a telescoped accumulation

    out_s(m) = wp1 + Rt(wq_1)K_2 + sum_{j=2}^{m-1} Rt(wq_j) G_j
             + Rt(wq_m)(sp_s - jp_m) + CONST_s

where Rt(q)v = R(q)v - v is the pure-quadratic part of the rotation and all
constant vectors (G_j, CONST_s) are host-precomputed from the tiny tree
tensors and baked into instruction immediates. The kernel is recompiled (and
disk-cached) per unique set of tree constants; qpos is the only streamed
input.
"""
import hashlib
import numpy as np

import concourse.bacc as bacc
import concourse.mybir as mybir
from concourse.tile import TileContext
from concourse.bass_utils import run_bass_kernel_spmd

F32 = mybir.dt.float32
MULT = mybir.AluOpType.mult
ADD = mybir.AluOpType.add
SUB = mybir.AluOpType.subtract

B_FULL = 131072
NCORES = 8
N = B_FULL // NCORES          # 16384 rows per core
P = 128                       # partitions
F = N // P                    # 128 free
NBODY = 63
NH = NBODY - 2                # 61
NQ = 7 + NH                   # 68
NSITES = 16
OUTW = NSITES * 3             # 48

_CACHE = {}


def _qmul_np(q1, q2):
    w1, x1, y1, z1 = [q1[..., i] for i in range(4)]
    w2, x2, y2, z2 = [q2[..., i] for i in range(4)]
    return np.stack([
        w1 * w2 - x1 * x2 - y1 * y2 - z1 * z2,
        w1 * x2 + x1 * w2 + y1 * z2 - z1 * y2,
        w1 * y2 - x1 * z2 + y1 * w2 + z1 * x2,
        w1 * z2 + x1 * y2 - y1 * x2 + z1 * w2,
    ], -1)


def _build(consts_key, body_pos, body_quat, hinge_axis, jnt_pos, site_pos,
           body_parent, site_body):
    # ---- host constant precompute ----
    parents = np.asarray(body_parent).astype(np.int64)
    sbody = np.asarray(site_body).astype(np.int64)
    assert np.array_equal(parents, np.maximum(np.arange(NBODY) - 1, 0)), \
        "kernel specialized for chain topology"

    A = np.asarray(body_quat[2:], np.float64)                      # [NH,4]
    Bq = _qmul_np(np.asarray(body_quat[2:], np.float64),
                  np.concatenate([np.zeros((NH, 1)), np.asarray(hinge_axis, np.float64)], -1))
    K = np.asarray(body_pos[2:], np.float64) + np.asarray(jnt_pos, np.float64)  # K_m, m=2..62
    jp = np.asarray(jnt_pos, np.float64)
    sp = np.asarray(site_pos, np.float64)

    G = np.zeros((NH, 3))
    for h in range(NH):
        G[h] = (K[h + 1] if h + 1 < NH else 0.0) - jp[h]
    # constant prefix C_m = K_2 + sum_{j=2}^{m-1} G_j
    Cpre = np.zeros((NBODY, 3))
    acc = K[0].copy()
    for m in range(2, NBODY):
        Cpre[m] = acc
        acc = acc + G[m - 2]

    site_by_body = {}
    for s, m in enumerate(sbody):
        site_by_body.setdefault(int(m), []).append(s)
    m_max = max(site_by_body.keys())

    # ---- bass program ----
    nc = bacc.Bacc("TRN2")
    qpos_d = nc.dram_tensor("qpos", [N, NQ], F32, kind="ExternalInput")
    out_d = nc.dram_tensor("sites", [N, OUTW], F32, kind="ExternalOutput")

    with TileContext(nc) as tc:
        with tc.tile_pool(name="main", bufs=1) as pool, \
             tc.tile_pool(name="scratch", bufs=2) as sp_pool:

            QP = pool.tile([P, F * NQ], F32)
            nc.sync.dma_start(QP[:], qpos_d[:].rearrange("(p f) k -> p (f k)", p=P))
            QPr = QP[:].rearrange("p (f k) -> p k f", k=NQ)   # [P, 68, F] strided view

            s_all = pool.tile([P, NH * F], F32)
            c_all = pool.tile([P, NH * F], F32)
            scr = pool.tile([P, NH * F], F32)
            ang = QPr[:, 7:NQ, :]                              # [P, 61, F]
            s3 = s_all[:].rearrange("p (h f) -> p h f", h=NH)
            c3 = c_all[:].rearrange("p (h f) -> p h f", h=NH)
            x3 = scr[:].rearrange("p (h f) -> p h f", h=NH)
            Sin, Square, Sqrt = (mybir.ActivationFunctionType.Sin,
                                 mybir.ActivationFunctionType.Square,
                                 mybir.ActivationFunctionType.Sqrt)
            # order matters: s_all doubles as the sin(th/4)^2 scratch first
            nc.scalar.activation(x3, ang, Sin, bias=0.0, scale=0.25)
            nc.scalar.activation(s3, x3, Square, bias=0.0, scale=1.0)
            # c = 1 - 2*sin^2(theta/4)
            nc.vector.tensor_scalar(c3, s3, -2.0, 1.0, MULT, ADD)
            nc.scalar.activation(s3, ang, Sin, bias=0.0, scale=0.5)

            def tile(tag):
                return sp_pool.tile([P, F], F32, tag=tag, name=tag)

            # ---- root ----
            rq = [QPr[:, 3 + i, :] for i in range(4)]
            n2 = tile("n2")
            nc.vector.tensor_tensor(n2, rq[0], rq[0], MULT)
            for i in range(1, 4):
                t = tile("rt")
                nc.vector.tensor_tensor(t, rq[i], rq[i], MULT)
                nc.vector.tensor_tensor(n2, n2, t, ADD)
            rn = sp_pool.tile([P, F], F32, tag="rn", name="rn")
            nc.scalar.activation(rn, n2, Sqrt, bias=0.0, scale=1.0)
            nc.vector.reciprocal(rn, rn)
            wq = [sp_pool.tile([P, F], F32, tag=f"wq{i}", name=f"wq{i}") for i in range(4)]
            for i in range(4):
                nc.vector.tensor_tensor(wq[i], rq[i], rn, MULT)

            Pacc = [pool.tile([P, F], F32, name=f"Pacc{i}") for i in range(3)]
            for i in range(3):
                nc.vector.tensor_copy(Pacc[i], QPr[:, i, :])

            OS = pool.tile([P, F * OUTW], F32)
            OSr = OS[:].rearrange("p (f k) -> p k f", k=OUTW)  # [P, 48, F]

            def emit_rot_sites_and_G(q, body, vecs):
                """q = [w,x,y,z] tiles. vecs = list of (v3, const3_or_None, out_idx).
                For each vec: if out_idx is None -> P += Rt(q)v  (position update)
                else OSr[:, out_idx..] = P + Rt(q)v + const (site emit, 3 comps).
                """
                w, u = q[0], q[1:4]
                for (v, cst, oidx) in vecs:
                    v2 = [2.0 * float(v[i]) for i in range(3)]
                    tx, ty, tz = tile("tx"), tile("ty"), tile("tz")
                    # t = cross(u, 2v)
                    for (to, ia, ib, ca, cb) in ((tx, 1, 2, v2[2], v2[1]),
                                                 (ty, 2, 0, v2[0], v2[2]),
                                                 (tz, 0, 1, v2[1], v2[0])):
                        m = tile("m")
                        nc.vector.tensor_scalar(m, u[ia], ca, None, MULT)
                        nc.vector.scalar_tensor_tensor(to, u[ib], -cb, m, MULT, ADD)
                    tv = (tx, ty, tz)
                    # r_i = w*t_i + (u x t)_i ; then P+= or site out
                    for ci, (ia, ib) in enumerate(((1, 2), (2, 0), (0, 1))):
                        a1 = tile("a1"); a2 = tile("a2"); a4 = tile("a4")
                        nc.vector.tensor_tensor(a1, w, tv[ci], MULT)
                        nc.vector.tensor_tensor(a2, u[ia], tv[ib], MULT)
                        nc.vector.tensor_tensor(a1, a1, a2, ADD)
                        nc.vector.tensor_tensor(a4, u[ib], tv[ia], MULT)
                        nc.vector.tensor_tensor(a1, a1, a4, SUB)
                        if oidx is None:
                            nc.vector.tensor_tensor(Pacc[ci], Pacc[ci], a1, ADD)
                        else:
                            nc.vector.scalar_tensor_tensor(
                                OSr[:, oidx + ci, :], a1, float(cst[ci]),
                                Pacc[ci], ADD, ADD)

            # sites on body 1 (root): out = wp1 + Rt(wq1) sp + sp, using Pacc==wp1
            for sid in site_by_body.get(1, []):
                emit_rot_sites_and_G(wq, 1, [(sp[sid], sp[sid], 3 * sid)])

            # P init: += Rt(wq1) K_2
            emit_rot_sites_and_G(wq, 1, [(K[0], None, None)])

            # ---- chain ----
            for j in range(2, m_max + 1):
                h = j - 2
                sh = s3[:, h, :]
                ch = c3[:, h, :]
                # lq = c*A + s*B
                lq = []
                for i in range(4):
                    ti = tile("lqt")
                    nc.vector.tensor_scalar(ti, sh, float(Bq[h, i]), None, MULT)
                    li = sp_pool.tile([P, F], F32, tag=f"lq{i}", name=f"lq{i}")
                    nc.vector.scalar_tensor_tensor(li, ch, float(A[h, i]), ti, MULT, ADD)
                    lq.append(li)
                # nq = wq x lq : 16 products
                pr = {}
                for a in range(4):
                    for b in range(4):
                        pab = sp_pool.tile([P, F], F32, tag=f"pr{a}{b}", name=f"pr{a}{b}")
                        nc.vector.tensor_tensor(pab, wq[a], lq[b], MULT)
                        pr[(a, b)] = pab
                nq = [sp_pool.tile([P, F], F32, tag=f"wq{i}", name=f"nq{i}") for i in range(4)]
                combos = [
                    (0, (0, 0), [((1, 1), SUB), ((2, 2), SUB), ((3, 3), SUB)]),
                    (1, (0, 1), [((1, 0), ADD), ((2, 3), ADD), ((3, 2), SUB)]),
                    (2, (0, 2), [((1, 3), SUB), ((2, 0), ADD), ((3, 1), ADD)]),
                    (3, (0, 3), [((1, 2), ADD), ((2, 1), SUB), ((3, 0), ADD)]),
                ]
                for (i, first, rest) in combos:
                    nc.vector.tensor_tensor(nq[i], pr[first], pr[rest[0][0]], rest[0][1])
                    for (key, op) in rest[1:]:
                        nc.vector.tensor_tensor(nq[i], nq[i], pr[key], op)
                wq = nq
                # sites on body j (before G update), then G update
                vecs = []
                for sid in site_by_body.get(j, []):
                    v = sp[sid] - jp[h]
                    vecs.append((v, Cpre[j] + v, 3 * sid))
                if j < m_max:
                    vecs.append((G[h], None, None))
                emit_rot_sites_and_G(wq, j, vecs)

            nc.sync.dma_start(out_d[:].rearrange("(p f) k -> p (f k)", p=P), OS[:])

    nc.compile()
    return nc


def _get_nc(inputs):
    key_src = b"".join(np.ascontiguousarray(np.asarray(inputs[k])).tobytes()
                       for k in ("body_pos", "body_quat", "hinge_axis", "jnt_pos",
                                 "site_pos", "body_parent", "site_body"))
    key = hashlib.sha256(key_src).hexdigest()
    if key not in _CACHE:
        _CACHE[key] = _build(key, inputs["body_pos"], inputs["body_quat"],
                             inputs["hinge_axis"], inputs["jnt_pos"],
                             inputs["site_pos"], inputs["body_parent"],
                             inputs["site_body"])
    return _CACHE[key]


def kernel(**inputs) -> np.ndarray:
    qpos = np.ascontiguousarray(np.asarray(inputs["qpos"], dtype=np.float32))
    assert qpos.shape == (B_FULL, NQ)
    nc = _get_nc(inputs)
    in_maps = [{"qpos": qpos[c * N:(c + 1) * N]} for c in range(NCORES)]
    res = run_bass_kernel_spmd(nc, in_maps, list(range(NCORES)))
    out = np.concatenate([res.results[c]["sites"] for c in range(NCORES)], axis=0)
    return out.reshape(B_FULL, NSITES, 3)


if __name__ == "__main__":
    rng = np.random.RandomState(0)
    import importlib.util
    spec = importlib.util.spec_from_file_location("reference", "/root/problem/reference.py")
    ref = importlib.util.module_from_spec(spec)
    spec.loader.exec_module(ref)
    inputs = {k: np.asarray(v) for k, v in ref.setup_inputs().items()}
    out = kernel(**inputs)
    print("out", out.shape, out.dtype)
